# revision 1
# baseline (speedup 1.0000x reference)
"""Multi-head self-attention TRN2 Bass kernel.

B=4, N=2048, C=1024, H=16, D=64 on 8 NeuronCores: core c handles batch
b=c//2 and head-group g=c%2 (8 heads); proj is row-parallel, partials
summed on host. All on-device compute in "transposed land" (no
transposes): q/k feature-major, v token-major with a ones column per head
(row 65 of the AV accumulation = softmax Z), score head-pairs packed in
one [128,1024] PSUM tile on disjoint PE row groups, one exp per tile,
av evacuated to SBUF immediately (frees the PSUM accumulator), normalize
(1/Z via DVE + GpSimd partition-broadcast) entirely off the critical
path; v tiles and the output projection stream through attention as
PE fillers; bf16 output partials.

Problem: B=4, N=2048, C=1024, H=16 heads, D=64. 8 NeuronCores.
Sharding: core c handles batch b=c//2, head-group g=c%2 (8 heads each).
Data parallel on B, tensor parallel on heads; proj is row-parallel with the
partial sums combined on the host.

All on-device compute stays in "transposed land" (no transposes needed):
  - host feeds x^T [1024, 2048] bf16; q/k biases are added during the
    PSUM->SBUF copy (per-partition tensor_scalar add, features are the
    partition dim); the v bias row is partition-broadcast once on GpSimd
    and added during the v copy
  - q^T,k^T computed feature-major [feat, tok]; v token-major [tok, feat]
    with a ones column interleaved per head (row 65 of AV = softmax Z)
  - scores for a head PAIR share one [128, 1024] PSUM tile (h0 cols 0:512,
    h1 cols 512:1024, nq block = 512); the two K=64 matmuls land on
    disjoint PE row groups (tile_position auto-derived) so they overlap
  - one exp per score tile on ScalarE (max-subtraction skipped: scores are
    ~N(0,0.33), safely inside fp32 exp range)
  - AV accumulates [65, 512] per head over 16 nk chunks; row 64 = Z
  - normalize: 1/Z (DVE) -> GpSimd partition_broadcast -> multiply
  - proj = matmul(lhsT=Wp^T, rhs=o_norm^T) -> out^T partial, fp32 to HBM

Emission order: qk(hp0), then attention hp0 BEFORE the v phase (AVs
naturally throttle on v tiles; exps start ~20us in), then qk(hp1),
attention hp1, ..., proj last (overlaps attention tail).
"""

import numpy as np
import ml_dtypes
from contextlib import ExitStack

N_CORES = 8
B, N, C = 4, 2048, 1024
H, D = 16, 64
HL = H // 2          # heads per core (8)
CL = HL * D          # local features per head-group (512)
KC = 8               # contraction chunks of 128 (= C/128, no ones row)
NKC = 16             # nk chunks of 128
NQB = 4              # nq blocks of 512
BF = ml_dtypes.bfloat16

_CACHE = {}


def _build(loop_n=1, staggered=False):
    import concourse.tile as tile
    from concourse import bacc, mybir

    bf = mybir.dt.bfloat16
    f32 = mybir.dt.float32
    AF = mybir.ActivationFunctionType

    nc = bacc.Bacc("TRN2", target_bir_lowering=False, debug=False,
                   num_devices=N_CORES)
    xT = nc.dram_tensor("xT", [C, N], bf, kind="ExternalInput").ap()
    wqk = nc.dram_tensor("wqk", [C, 2 * CL], bf, kind="ExternalInput").ap()
    wv = nc.dram_tensor("wv", [C, CL], bf, kind="ExternalInput").ap()
    wp = nc.dram_tensor("wp", [CL, C], bf, kind="ExternalInput").ap()
    qkb = nc.dram_tensor("qkb", [128, 8], f32, kind="ExternalInput").ap()
    vb = nc.dram_tensor("vb", [1, CL], bf, kind="ExternalInput").ap()
    outT = nc.dram_tensor("outT", [C, N], bf, kind="ExternalOutput").ap()

    xT_r = xT.rearrange("(k p) n -> k p n", p=128)
    wqk_r = wqk.rearrange("(k p) n -> k p n", p=128)
    wv_r = wv.rearrange("(k p) n -> k p n", p=128)
    wp_r = wp.rearrange("(k p) n -> k p n", p=128)

    with tile.TileContext(nc) as tc, ExitStack() as ctx:
        const = ctx.enter_context(tc.tile_pool(name="const", bufs=1))
        x_sb = const.tile([128, KC, N], bf)
        wqk_sb = const.tile([128, KC, 2 * CL], bf)
        wv_sb = const.tile([128, KC, CL], bf)
        wp_sb = const.tile([128, 4, C], bf)
        qkb_sb = const.tile([128, 8], f32)
        vb_sb = const.tile([1, CL], bf)
        vbb_sb = const.tile([128, CL], bf)          # v bias broadcast
        qk_sb = const.tile([128, 8, N], bf)         # [feat%128, feat_tile, tok]
        v_sb = const.tile([128, NKC, HL * 65], bf)  # v w/ ones col per head
        o_sb = const.tile([128, 4, N], bf)          # o_norm^T [cloc%128, chunk, tok]

        p_pool = ctx.enter_context(tc.tile_pool(name="p", bufs=6))
        avsb_pool = ctx.enter_context(tc.tile_pool(name="avsb", bufs=4))
        bcs_pool = ctx.enter_context(tc.tile_pool(name="bcs", bufs=4))
        norm_pool = ctx.enter_context(tc.tile_pool(name="norm", bufs=4))
        ostage_pool = ctx.enter_context(tc.tile_pool(name="ostage", bufs=4))

        # PSUM: 8 banks. s: 2 bufs x [128,1024]f32 (2 banks) = 4.
        # av: 2 bufs x [65,512]f32 (1 bank) = 2.
        # mm (qk/v/proj shared): 2 bufs x [128,512]f32 (1 bank) = 2.
        sps = ctx.enter_context(tc.tile_pool(name="sps", bufs=2, space="PSUM"))
        avps = ctx.enter_context(tc.tile_pool(name="avps", bufs=2, space="PSUM"))
        mmps = ctx.enter_context(tc.tile_pool(name="mmps", bufs=2, space="PSUM"))

        def _loads():
            # interleave DMAs so qk(hp0) inputs land first
            nc.sync.dma_start(qkb_sb[:], qkb)
            nc.sync.dma_start(vb_sb[:], vb)
            for k in range(KC):
                nc.sync.dma_start(x_sb[:, k, :], xT_r[k])
                nc.sync.dma_start(wqk_sb[:, k, :], wqk_r[k])
            for k in range(KC):
                nc.sync.dma_start(wv_sb[:, k, :], wv_r[k])
            for k in range(4):
                nc.sync.dma_start(wp_sb[:, k, :], wp_r[k])
            nc.gpsimd.partition_broadcast(vbb_sb[:], vb_sb[0:1, :])
            v_ones = v_sb.rearrange(
                "p t (h e) -> p t h e", e=65)[:, :, :, 64:65]
            nc.vector.memset(v_ones, 1.0)

        def _qk_unit(ft, tp):
            # one (feature tile, token-block pair) of q^T / k^T,
            # feature-major [feat 128, tok]; the tb pair shares the
            # stationary weight chunk (LDW amortized 2x). Bias lands in
            # the PSUM->SBUF copy (per-partition scalar add).
            pss = [mmps.tile([128, 512], f32, tag="mm",
                             name="qk%d" % i) for i in range(2)]
            for k in range(KC):
                for i in range(2):
                    tb = tp * 2 + i
                    nc.tensor.matmul(
                        pss[i][:],
                        wqk_sb[:, k, ft * 128:(ft + 1) * 128],
                        x_sb[:, k, tb * 512:(tb + 1) * 512],
                        start=(k == 0), stop=(k == KC - 1),
                    )
            for i in range(2):
                tb = tp * 2 + i
                nc.vector.tensor_add(
                    qk_sb[:, ft, tb * 512:(tb + 1) * 512], pss[i][:],
                    qkb_sb[:, ft:ft + 1].broadcast_to([128, 512]))

        def _qk_pair(hp):
            for ft in (hp, 4 + hp):
                for tp in range(2):
                    _qk_unit(ft, tp)

        def _v_tile(tt):
            # v token-major [tok 128, feat 512] (AV lhsT layout); bias
            # added during the copy via the broadcast bias tile
            ps = mmps.tile([128, 512], f32, tag="mm")
            for k in range(KC):
                nc.tensor.matmul(
                    ps[:],
                    x_sb[:, k, tt * 128:(tt + 1) * 128],
                    wv_sb[:, k, :],
                    start=(k == 0), stop=(k == KC - 1),
                )
            v_out = v_sb[:, tt, :].rearrange(
                "p (h e) -> p h e", e=65)[:, :, 0:64]
            v_in = ps[:].rearrange("p (h e) -> p h e", e=64)
            vb_in = vbb_sb.rearrange("p (h e) -> p h e", e=64)
            nc.vector.tensor_add(v_out, v_in, vb_in)

        def _attn(hp, ck_filler=None, nqb_filler=None):
            h0, h1 = 2 * hp, 2 * hp + 1
            for nqb in range(NQB):
                q0 = nqb * 512
                avA = avps.tile([65, 512], f32, tag="av")
                avB = avps.tile([65, 512], f32, tag="av")
                for ck in range(NKC):
                    s = sps.tile([128, 1024], f32, tag="s")
                    kslc = slice(ck * 128, (ck + 1) * 128)
                    qslc = slice(q0, q0 + 512)
                    # two K=64 matmuls on disjoint PE row groups (auto
                    # tile_position (0,0) / (64,0)), disjoint PSUM banks
                    nc.tensor.matmul(
                        s[:, 0:512],
                        qk_sb[0:64, 4 + hp, kslc],
                        qk_sb[0:64, hp, qslc], start=True, stop=True)
                    nc.tensor.matmul(
                        s[:, 512:1024],
                        qk_sb[64:128, 4 + hp, kslc],
                        qk_sb[64:128, hp, qslc], start=True, stop=True)
                    p = p_pool.tile([128, 1024], bf, tag="p")
                    nc.scalar.activation(p[:], s[:], AF.Exp)
                    nc.tensor.matmul(
                        avA[:],
                        v_sb[:, ck, h0 * 65:h0 * 65 + 65],
                        p[:, 0:512],
                        start=(ck == 0), stop=(ck == NKC - 1))
                    nc.tensor.matmul(
                        avB[:],
                        v_sb[:, ck, h1 * 65:h1 * 65 + 65],
                        p[:, 512:1024],
                        start=(ck == 0), stop=(ck == NKC - 1))
                    if nqb == 0 and ck_filler is not None:
                        ck_filler(ck)
                # evacuate av PSUM -> SBUF immediately (frees the PSUM
                # accumulator for the next nq block), then normalize from
                # SBUF entirely off the AV critical path:
                # o = av[0:64] * (1/Z), Z = av row 64; 1/Z broadcast across
                # partitions on the (idle) GpSimd.
                for lh, av in ((h0, avA), (h1, avB)):
                    av_sb = avsb_pool.tile([65, 512], bf, tag="avsb")
                    with nc.allow_low_precision(
                            reason="av/Z staged bf16; validated 3e-3 e2e"):
                        nc.vector.tensor_copy(av_sb[:], av[:])
                    recip = norm_pool.tile([1, 512], bf, tag="recip")
                    with nc.allow_low_precision(
                            reason="1/Z in bf16; validated 2e-3 e2e"):
                        nc.vector.reciprocal(recip[:], av_sb[64:65, :])
                    bc_sb = bcs_pool.tile([64, 512], bf, tag="bc")
                    nc.gpsimd.partition_broadcast(bc_sb[:], recip[0:1, :])
                    part = slice(0, 64) if lh % 2 == 0 else slice(64, 128)
                    nc.vector.tensor_mul(
                        o_sb[part, lh // 2, q0:q0 + 512],
                        av_sb[0:64, :], bc_sb[:])
                if nqb_filler is not None:
                    nqb_filler(nqb)

        def _proj_block(nqb):
            # out^T [C, N] partial for one nq block
            for ct in range(8):
                ps = mmps.tile([128, 512], f32, tag="mm")
                for k in range(4):
                    nc.tensor.matmul(
                        ps[:],
                        wp_sb[:, k, ct * 128:(ct + 1) * 128],
                        o_sb[:, k, nqb * 512:(nqb + 1) * 512],
                        start=(k == 0), stop=(k == 3),
                    )
                ostage = ostage_pool.tile([128, 512], bf, tag="o")
                nc.vector.tensor_copy(ostage[:], ps[:])
                nc.sync.dma_start(
                    outT[ct * 128:(ct + 1) * 128,
                         nqb * 512:(nqb + 1) * 512],
                    ostage[:])

        def _body():
            _loads()
            _qk_pair(0)
            for tt in range(2):
                _v_tile(tt)
            # remaining v tiles interleave into attn(0)'s first nq block,
            # two tiles ahead of the AV chunk that consumes them
            _attn(0, ck_filler=lambda ck: _v_tile(ck + 2) if ck < 14 else None)
            for hp in range(1, 4):
                _qk_pair(hp)
                if hp < 3:
                    _attn(hp)
            _attn(3, nqb_filler=_proj_block)

        if loop_n > 1:
            with tc.For_i(0, loop_n, 1,
                          staggered_reset=staggered,
                          hint_engines=(mybir.EngineType.PE,
                                        mybir.EngineType.Activation,
                                        mybir.EngineType.DVE,
                                        mybir.EngineType.Pool,
                                        mybir.EngineType.SP)):
                _body()
        else:
            _body()

    nc.compile()
    return nc


def _prep_core_inputs(x, w_qkv, b_qkv, w_proj, core):
    b, g = core // 2, core % 2
    scale = np.float32(D) ** -0.5

    xT = np.ascontiguousarray(x[b].T).astype(BF)

    q_w = w_qkv[g * CL:(g + 1) * CL] * scale
    k_w = w_qkv[C + g * CL:C + (g + 1) * CL]
    v_w = w_qkv[2 * C + g * CL:2 * C + (g + 1) * CL]
    q_b = b_qkv[g * CL:(g + 1) * CL] * scale
    k_b = b_qkv[C + g * CL:C + (g + 1) * CL]
    v_b = b_qkv[2 * C + g * CL:2 * C + (g + 1) * CL]

    wqk = np.empty((C, 2 * CL), dtype=BF)
    wqk[:, :CL] = q_w.T.astype(BF)
    wqk[:, CL:] = k_w.T.astype(BF)

    # q/k biases as per-partition scalars: [feat%128, feat_tile]
    qkb = np.ascontiguousarray(
        np.concatenate([q_b, k_b]).astype(np.float32).reshape(8, 128).T)

    return {"xT": xT, "wqk": wqk, "wv": v_w.T.astype(BF).copy(),
            "wp": np.ascontiguousarray(
                w_proj[:, g * CL:(g + 1) * CL].T).astype(BF),
            "qkb": qkb, "vb": v_b.astype(BF).reshape(1, CL)}


def kernel(x, w_qkv, b_qkv, w_proj, b_proj):
    from concourse.bass_utils import run_bass_kernel_spmd

    x = np.asarray(x, dtype=np.float32)
    w_qkv = np.asarray(w_qkv, dtype=np.float32)
    b_qkv = np.asarray(b_qkv, dtype=np.float32)
    w_proj = np.asarray(w_proj, dtype=np.float32)
    b_proj = np.asarray(b_proj, dtype=np.float32)

    if "nc" not in _CACHE:
        _CACHE["nc"] = _build()
    nc = _CACHE["nc"]

    in_maps = [_prep_core_inputs(x, w_qkv, b_qkv, w_proj, c)
               for c in range(N_CORES)]
    res = run_bass_kernel_spmd(nc, in_maps, core_ids=list(range(N_CORES)))
    _CACHE["last_results"] = res

    out = np.empty((B, N, C), dtype=np.float32)
    for b in range(B):
        acc = (res.results[2 * b]["outT"].astype(np.float32)
               + res.results[2 * b + 1]["outT"].astype(np.float32))
        out[b] = acc.T + b_proj[None, :]
    return out



# revision 3
# speedup vs baseline: 1.6807x; 1.6807x over previous
"""Multi-head self-attention TRN2 Bass kernel.

B=4, N=2048, C=1024, H=16, D=64 on 8 NeuronCores: core c handles batch
b=c//2 and head-group g=c%2 (8 heads); proj is row-parallel, partials
summed on host. All on-device compute in "transposed land" (no
transposes): q/k feature-major, v token-major with a ones column per head
(row 65 of the AV accumulation = softmax Z), score head-pairs packed in
one [128,1024] PSUM tile on disjoint PE row groups, one exp per tile,
av evacuated to SBUF immediately (frees the PSUM accumulator), normalize
(1/Z via DVE + GpSimd partition-broadcast) entirely off the critical
path; v tiles and the output projection stream through attention as
PE fillers; bf16 output partials.

Problem: B=4, N=2048, C=1024, H=16 heads, D=64. 8 NeuronCores.
Sharding: core c handles batch b=c//2, head-group g=c%2 (8 heads each).
Data parallel on B, tensor parallel on heads; proj is row-parallel with the
partial sums combined on the host.

All on-device compute stays in "transposed land" (no transposes needed):
  - host feeds x^T [1024, 2048] bf16; q/k biases are added during the
    PSUM->SBUF copy (per-partition tensor_scalar add, features are the
    partition dim); the v bias row is partition-broadcast once on GpSimd
    and added during the v copy
  - q^T,k^T computed feature-major [feat, tok]; v token-major [tok, feat]
    with a ones column interleaved per head (row 65 of AV = softmax Z)
  - scores for a head PAIR share one [128, 1024] PSUM tile (h0 cols 0:512,
    h1 cols 512:1024, nq block = 512); the two K=64 matmuls land on
    disjoint PE row groups (tile_position auto-derived) so they overlap
  - one exp per score tile on ScalarE (max-subtraction skipped: scores are
    ~N(0,0.33), safely inside fp32 exp range)
  - AV accumulates [65, 512] per head over 16 nk chunks; row 64 = Z
  - normalize: 1/Z (DVE) -> GpSimd partition_broadcast -> multiply
  - proj = matmul(lhsT=Wp^T, rhs=o_norm^T) -> out^T partial, fp32 to HBM

Emission order: qk(hp0), then attention hp0 BEFORE the v phase (AVs
naturally throttle on v tiles; exps start ~20us in), then qk(hp1),
attention hp1, ..., proj last (overlaps attention tail).
"""

import numpy as np
import ml_dtypes
from contextlib import ExitStack

N_CORES = 8
B, N, C = 4, 2048, 1024
H, D = 16, 64
HL = H // 2          # heads per core (8)
CL = HL * D          # local features per head-group (512)
KC = 8               # contraction chunks of 128 (= C/128, no ones row)
NKC = 16             # nk chunks of 128
NQB = 4              # nq blocks of 512
BF = ml_dtypes.bfloat16

_CACHE = {}


def _build(loop_n=1, staggered=False):
    import concourse.tile as tile
    from concourse import bacc, mybir

    bf = mybir.dt.bfloat16
    f32 = mybir.dt.float32
    AF = mybir.ActivationFunctionType

    nc = bacc.Bacc("TRN2", target_bir_lowering=False, debug=False,
                   num_devices=N_CORES)
    xT = nc.dram_tensor("xT", [C, N], bf, kind="ExternalInput").ap()
    wqk = nc.dram_tensor("wqk", [C, 2 * CL], bf, kind="ExternalInput").ap()
    wv = nc.dram_tensor("wv", [C, CL], bf, kind="ExternalInput").ap()
    wp = nc.dram_tensor("wp", [CL, C], bf, kind="ExternalInput").ap()
    qkb = nc.dram_tensor("qkb", [128, 8], f32, kind="ExternalInput").ap()
    vb = nc.dram_tensor("vb", [1, CL], bf, kind="ExternalInput").ap()
    outT = nc.dram_tensor("outT", [C, N], bf, kind="ExternalOutput").ap()

    xT_r = xT.rearrange("(k p) n -> k p n", p=128)
    wqk_r = wqk.rearrange("(k p) n -> k p n", p=128)
    wv_r = wv.rearrange("(k p) n -> k p n", p=128)
    wp_r = wp.rearrange("(k p) n -> k p n", p=128)

    with tile.TileContext(nc) as tc, ExitStack() as ctx:
        const = ctx.enter_context(tc.tile_pool(name="const", bufs=1))
        x_sb = const.tile([128, KC, N], bf)
        wqk_sb = const.tile([128, KC, 2 * CL], bf)
        wv_sb = const.tile([128, KC, CL], bf)
        wp_sb = const.tile([128, 4, C], bf)
        qkb_sb = const.tile([128, 8], f32)
        vb_sb = const.tile([1, CL], bf)
        vbb_sb = const.tile([128, CL], bf)          # v bias broadcast
        qk_sb = const.tile([128, 8, N], bf)         # [feat%128, feat_tile, tok]
        v_sb = const.tile([128, NKC, HL * 65], bf)  # v w/ ones col per head
        o_sb = const.tile([128, 4, N], bf)          # o_norm^T [cloc%128, chunk, tok]

        p_pool = ctx.enter_context(tc.tile_pool(name="p", bufs=6))
        avsb_pool = ctx.enter_context(tc.tile_pool(name="avsb", bufs=4))
        bcs_pool = ctx.enter_context(tc.tile_pool(name="bcs", bufs=4))
        norm_pool = ctx.enter_context(tc.tile_pool(name="norm", bufs=4))
        ostage_pool = ctx.enter_context(tc.tile_pool(name="ostage", bufs=4))

        # PSUM: 8 banks. s: 2 bufs x [128,1024]f32 (2 banks) = 4.
        # av: 2 bufs x [65,512]f32 (1 bank) = 2.
        # mm (qk/v/proj shared): 2 bufs x [128,512]f32 (1 bank) = 2.
        sps = ctx.enter_context(tc.tile_pool(name="sps", bufs=2, space="PSUM"))
        avps = ctx.enter_context(tc.tile_pool(name="avps", bufs=2, space="PSUM"))
        mmps = ctx.enter_context(tc.tile_pool(name="mmps", bufs=2, space="PSUM"))

        def _loads():
            # interleave DMAs so qk(hp0) inputs land first
            nc.sync.dma_start(qkb_sb[:], qkb)
            nc.sync.dma_start(vb_sb[:], vb)
            for k in range(KC):
                nc.sync.dma_start(x_sb[:, k, :], xT_r[k])
                nc.sync.dma_start(wqk_sb[:, k, :], wqk_r[k])
            for k in range(KC):
                nc.sync.dma_start(wv_sb[:, k, :], wv_r[k])
            for k in range(4):
                nc.sync.dma_start(wp_sb[:, k, :], wp_r[k])
            nc.gpsimd.partition_broadcast(vbb_sb[:], vb_sb[0:1, :])
            v_ones = v_sb.rearrange(
                "p t (h e) -> p t h e", e=65)[:, :, :, 64:65]
            nc.vector.memset(v_ones, 1.0)

        def _qk_unit(ft, tp):
            # one (feature tile, token-block pair) of q^T / k^T,
            # feature-major [feat 128, tok]; the tb pair shares the
            # stationary weight chunk (LDW amortized 2x). Bias lands in
            # the PSUM->SBUF copy (per-partition scalar add). Generator:
            # yields every 4 matmuls so filler work interleaves finely
            # with the attention chain (ACT must never starve).
            pss = [mmps.tile([128, 512], f32, tag="mm",
                             name="qk%d" % i) for i in range(2)]
            for k in range(KC):
                for i in range(2):
                    tb = tp * 2 + i
                    nc.tensor.matmul(
                        pss[i][:],
                        wqk_sb[:, k, ft * 128:(ft + 1) * 128],
                        x_sb[:, k, tb * 512:(tb + 1) * 512],
                        start=(k == 0), stop=(k == KC - 1),
                    )
                if k % 2 == 1 and k < KC - 1:
                    yield
            for i in range(2):
                tb = tp * 2 + i
                nc.vector.tensor_add(
                    qk_sb[:, ft, tb * 512:(tb + 1) * 512], pss[i][:],
                    qkb_sb[:, ft:ft + 1].broadcast_to([128, 512]))
            yield

        def _v_tile(tt):
            # v token-major [tok 128, feat 512] (AV lhsT layout); bias
            # added during the copy via the broadcast bias tile
            ps = mmps.tile([128, 512], f32, tag="mm")
            for k in range(KC):
                nc.tensor.matmul(
                    ps[:],
                    x_sb[:, k, tt * 128:(tt + 1) * 128],
                    wv_sb[:, k, :],
                    start=(k == 0), stop=(k == KC - 1),
                )
                if k == 3:
                    yield
            v_out = v_sb[:, tt, :].rearrange(
                "p (h e) -> p h e", e=65)[:, :, 0:64]
            v_in = ps[:].rearrange("p (h e) -> p h e", e=64)
            vb_in = vbb_sb.rearrange("p (h e) -> p h e", e=64)
            nc.vector.tensor_add(v_out, v_in, vb_in)
            yield

        def _proj_ct(nqb, ct):
            ps = mmps.tile([128, 512], f32, tag="mm")
            for k in range(4):
                nc.tensor.matmul(
                    ps[:],
                    wp_sb[:, k, ct * 128:(ct + 1) * 128],
                    o_sb[:, k, nqb * 512:(nqb + 1) * 512],
                    start=(k == 0), stop=(k == 3),
                )
            ostage = ostage_pool.tile([128, 512], bf, tag="o")
            nc.vector.tensor_copy(ostage[:], ps[:])
            nc.sync.dma_start(
                outT[ct * 128:(ct + 1) * 128,
                     nqb * 512:(nqb + 1) * 512],
                ostage[:])
            yield

        # filler queue: generators of small PE work chunks (~4 matmuls
        # per step) drained one step per attention ck slot, keeping PE
        # backlog dense without ever bursting ahead of the exp chain.
        fillq = []

        def _drain(steps=1):
            while steps > 0 and fillq:
                try:
                    next(fillq[0])
                    steps -= 1
                except StopIteration:
                    fillq.pop(0)

        def _run_all(g):
            for _ in g:
                pass

        def _attn(hp, steps_per_ck=1):
            h0, h1 = 2 * hp, 2 * hp + 1
            for nqb in range(NQB):
                q0 = nqb * 512
                avA = avps.tile([65, 512], f32, tag="av")
                avB = avps.tile([65, 512], f32, tag="av")
                for ck in range(NKC):
                    s = sps.tile([128, 1024], f32, tag="s")
                    kslc = slice(ck * 128, (ck + 1) * 128)
                    qslc = slice(q0, q0 + 512)
                    # two K=64 matmuls on disjoint PE row groups (auto
                    # tile_position (0,0) / (64,0)), disjoint PSUM banks
                    nc.tensor.matmul(
                        s[:, 0:512],
                        qk_sb[0:64, 4 + hp, kslc],
                        qk_sb[0:64, hp, qslc], start=True, stop=True)
                    nc.tensor.matmul(
                        s[:, 512:1024],
                        qk_sb[64:128, 4 + hp, kslc],
                        qk_sb[64:128, hp, qslc], start=True, stop=True)
                    p = p_pool.tile([128, 1024], bf, tag="p")
                    nc.scalar.activation(p[:], s[:], AF.Exp)
                    nc.tensor.matmul(
                        avA[:],
                        v_sb[:, ck, h0 * 65:h0 * 65 + 65],
                        p[:, 0:512],
                        start=(ck == 0), stop=(ck == NKC - 1))
                    nc.tensor.matmul(
                        avB[:],
                        v_sb[:, ck, h1 * 65:h1 * 65 + 65],
                        p[:, 512:1024],
                        start=(ck == 0), stop=(ck == NKC - 1))
                    if hp == 0 and nqb == 0:
                        # v tiles must stay 2 ahead of the AV chunk that
                        # consumes them; run a whole tile per slot
                        if ck < 14:
                            _run_all(_v_tile(ck + 2))
                    else:
                        _drain(steps_per_ck)
                # evacuate av PSUM -> SBUF immediately (frees the PSUM
                # accumulator for the next nq block), then normalize from
                # SBUF entirely off the AV critical path:
                # o = av[0:64] * (1/Z), Z = av row 64; 1/Z broadcast across
                # partitions on the (idle) GpSimd.
                for lh, av in ((h0, avA), (h1, avB)):
                    av_sb = avsb_pool.tile([65, 512], bf, tag="avsb")
                    with nc.allow_low_precision(
                            reason="av/Z staged bf16; validated 3e-3 e2e"):
                        nc.vector.tensor_copy(av_sb[:], av[:])
                    recip = norm_pool.tile([1, 512], bf, tag="recip")
                    with nc.allow_low_precision(
                            reason="1/Z in bf16; validated 2e-3 e2e"):
                        nc.vector.reciprocal(recip[:], av_sb[64:65, :])
                    bc_sb = bcs_pool.tile([64, 512], bf, tag="bc")
                    nc.gpsimd.partition_broadcast(bc_sb[:], recip[0:1, :])
                    part = slice(0, 64) if lh % 2 == 0 else slice(64, 128)
                    nc.vector.tensor_mul(
                        o_sb[part, lh // 2, q0:q0 + 512],
                        av_sb[0:64, :], bc_sb[:])
                if hp == 3:
                    # o_sb for this nq block is complete; spread its proj
                    # column tiles into the next nq block's ck slots (the
                    # last block's proj drains at the end)
                    for ct in range(8):
                        fillq.append(_proj_ct(nqb, ct))

        def _body():
            _loads()
            for ft in (0, 4):
                for tp in range(2):
                    _run_all(_qk_unit(ft, tp))
            for tt in range(2):
                _run_all(_v_tile(tt))
            # qk for head-pair hp+1 streams through attn(hp) as filler;
            # proj(nqb) streams through attn(3)'s following nq block
            fillq.extend(_qk_unit(ft, tp)
                         for ft in (1, 5) for tp in range(2))
            _attn(0)
            _drain(10 ** 9)  # qk(1) must be fully emitted before attn(1)
            fillq.extend(_qk_unit(ft, tp)
                         for ft in (2, 6) for tp in range(2))
            _attn(1)
            _drain(10 ** 9)
            fillq.extend(_qk_unit(ft, tp)
                         for ft in (3, 7) for tp in range(2))
            _attn(2)
            _drain(10 ** 9)
            _attn(3)
            _drain(10 ** 9)

        if loop_n > 1:
            with tc.For_i(0, loop_n, 1,
                          staggered_reset=staggered,
                          hint_engines=(mybir.EngineType.PE,
                                        mybir.EngineType.Activation,
                                        mybir.EngineType.DVE,
                                        mybir.EngineType.Pool,
                                        mybir.EngineType.SP)):
                _body()
        else:
            _body()

    nc.compile()
    return nc


def _prep_core_inputs(x, w_qkv, b_qkv, w_proj, core):
    b, g = core // 2, core % 2
    scale = np.float32(D) ** -0.5

    xT = np.ascontiguousarray(x[b].T).astype(BF)

    q_w = w_qkv[g * CL:(g + 1) * CL] * scale
    k_w = w_qkv[C + g * CL:C + (g + 1) * CL]
    v_w = w_qkv[2 * C + g * CL:2 * C + (g + 1) * CL]
    q_b = b_qkv[g * CL:(g + 1) * CL] * scale
    k_b = b_qkv[C + g * CL:C + (g + 1) * CL]
    v_b = b_qkv[2 * C + g * CL:2 * C + (g + 1) * CL]

    wqk = np.empty((C, 2 * CL), dtype=BF)
    wqk[:, :CL] = q_w.T.astype(BF)
    wqk[:, CL:] = k_w.T.astype(BF)

    # q/k biases as per-partition scalars: [feat%128, feat_tile]
    qkb = np.ascontiguousarray(
        np.concatenate([q_b, k_b]).astype(np.float32).reshape(8, 128).T)

    return {"xT": xT, "wqk": wqk, "wv": v_w.T.astype(BF).copy(),
            "wp": np.ascontiguousarray(
                w_proj[:, g * CL:(g + 1) * CL].T).astype(BF),
            "qkb": qkb, "vb": v_b.astype(BF).reshape(1, CL)}


def kernel(x, w_qkv, b_qkv, w_proj, b_proj):
    from concourse.bass_utils import run_bass_kernel_spmd

    x = np.asarray(x, dtype=np.float32)
    w_qkv = np.asarray(w_qkv, dtype=np.float32)
    b_qkv = np.asarray(b_qkv, dtype=np.float32)
    w_proj = np.asarray(w_proj, dtype=np.float32)
    b_proj = np.asarray(b_proj, dtype=np.float32)

    if "nc" not in _CACHE:
        _CACHE["nc"] = _build()
    nc = _CACHE["nc"]

    in_maps = [_prep_core_inputs(x, w_qkv, b_qkv, w_proj, c)
               for c in range(N_CORES)]
    res = run_bass_kernel_spmd(nc, in_maps, core_ids=list(range(N_CORES)))
    _CACHE["last_results"] = res

    out = np.empty((B, N, C), dtype=np.float32)
    for b in range(B):
        acc = (res.results[2 * b]["outT"].astype(np.float32)
               + res.results[2 * b + 1]["outT"].astype(np.float32))
        out[b] = acc.T + b_proj[None, :]
    return out

